# revision 6
# baseline (speedup 1.0000x reference)
"""MoE down-projection (grouped GEMM + topk combine) on 8 Trainium2 cores.

Strategy: expert-parallel. Each of the 8 cores owns E/8 = 16 experts and
receives (a) its experts' weight slabs and (b) the x rows routed to those
experts, gathered+gate-scaled+transposed on host, padded per expert to a
fixed capacity C. The device kernel is a block-diagonal grouped GEMM.
Weights stream through the PE as the moving operand (full rate); the few
x rows per expert are the stationary operand. NG = 128//C experts are
processed concurrently in separate PE column groups (tile_position), each
owning a contiguous C-partition range of a [128, H] PSUM tile, so output
stores are plain partition slices of only the real token rows. Host
scatter-adds the row results back into the [T, H] output.

The kernel is HBM-bandwidth bound on the weight stream, so the default
config stores w as fp8 E3M4 (per-expert scale folded into the x rows) and
the y output as bf16 — halving both read and write traffic vs bf16/f32 at
~1.3e-2 relative error (gate is 2e-2). Experts are assigned to (core,
slot) in count-sorted rank groups so one per-slot row count is tight for
the whole SPMD program.

Hardcoded problem shape (from the problem spec):
  x: [2048, 512] f32, w: [128, 512, 2048] f32,
  chosen_experts: [1024, 2] int, expert_weight: [1024, 2] f32 -> out [1024, 2048] f32
"""

import numpy as np

T = 1024
K_TOP = 2
E = 128
I_DIM = 512
H = 2048
N_CORES = 8
EPC = E // N_CORES  # experts per core = 16
P = 128             # partitions
I_CHUNKS = I_DIM // P       # 4
H_CHUNK = 512               # matmul moving free dim (fp32 PSUM bank)
H_CHUNKS = H // H_CHUNK     # 4

# matmul dtype config: name -> (w dtype, x dtype, y dtype)
#   float8e3  : w E3M4 (per-expert scaled), x bf16, y bf16 — half DMA traffic
#   float8e3x : both operands E3M4 (if mixed-dtype matmul is unsupported)
#   bfloat16  : both bf16, y f32
#   float32 / float32r: exact / reduced-precision f32
DT_CONFIGS = {
    "float8e3": ("float8e3", "bfloat16", "bfloat16"),
    "float8e3x": ("float8e3", "float8e3", "bfloat16"),
    "bfloat16": ("bfloat16", "bfloat16", "float32"),
    "float32": ("float32", "float32", "float32"),
    "float32r": ("float32r", "float32r", "float32"),
}
DEFAULT_DTYPE = "float8e3"
E3M4_SCALE_TARGET = 14.0  # keep clear of the 15.5 e3m4 max normal

_cache = {}


def _w_bytes(w_dtn):
    return 1 if w_dtn == "float8e3" else (2 if w_dtn == "bfloat16" else 4)


def _eps(w_dtn):
    """Experts per DMA slab. 2 MiB transfers amortize the per-DMA completion
    receipt (~1-2 us) that gates the next trigger on the same sem lane while
    keeping per-expert compute release fine-grained; bigger slabs quantize
    the compute pipeline and pay larger receipt latencies (measured loss)."""
    return max(1, 2 * 1024 * 1024 // (P * I_CHUNKS * H * _w_bytes(w_dtn)))


def _build(C: int, dt_name: str, ns: tuple | None = None):
    """ns: per-slot valid row counts (same for every core by construction —
    the host assigns experts to slots in count-sorted rank groups). When
    given, y stores move only those rows."""
    import concourse.mybir as mybir
    import concourse.tile as tile
    from concourse import bacc

    w_dtn, x_dtn, y_dtn = DT_CONFIGS[dt_name]
    w_dt = getattr(mybir.dt, w_dtn)
    x_dt = getattr(mybir.dt, x_dtn)
    y_dt = getattr(mybir.dt, y_dtn)
    w_bytes = _w_bytes(w_dtn)
    EPS = _eps(w_dtn)
    SLABS = EPC // EPS
    SLAB_COLS = EPS * I_CHUNKS * H
    # G = 128//C PE column groups run one expert's G h-chunks concurrently;
    # expert b's H chunk h goes to psum partitions (h%G)*C..+C, bank cols
    # (h//G)*512..+512, so casts and stores use all 128 partitions. Host
    # unpacks. fp32 rejects tile_position col-tiling.
    G = max(1, P // C)
    if H_CHUNKS % G != 0 or w_dtn not in ("bfloat16", "float8e3"):
        G = 1
    NB = H_CHUNKS // G
    PPART = G * C
    # keep the whole weight working set resident when it fits (fp8: 16 MiB)
    wbufs = SLABS if w_bytes == 1 else (6 if w_bytes == 2 else 3)
    obufs = 4 if w_bytes <= 2 else 2
    pbufs = 6 if NB == 1 else 2

    nc = bacc.Bacc()
    # wc host-prearranged: [k, p, e*ICH*H + i*H + h] = w[k*EPS+e, i*128+p, h]
    # so each partition's slab line is 1 contiguous run per DMA
    wc = nc.declare_dram_parameter("wc", [SLABS, P, SLAB_COLS], w_dt, isOutput=False)
    # x host-prearranged: [p, i*EC + c] = x[i*128+p, c] (EC = EPC*C) so the
    # whole stationary operand arrives in ONE small DMA before the w flood
    xT = nc.declare_dram_parameter("xT", [P, I_CHUNKS * EPC * C], x_dt, isOutput=False)
    y = nc.declare_dram_parameter("y", [EPC, PPART, NB * H_CHUNK], y_dt, isOutput=True)

    with tile.TileContext(nc) as tc:
        with (
            tc.tile_pool(name="wp", bufs=wbufs) as wp,
            tc.tile_pool(name="xp", bufs=1) as xp,
            tc.tile_pool(name="pp", bufs=pbufs, space="PSUM") as pp,
            tc.tile_pool(name="op", bufs=obufs) as op,
        ):
            # x rows (stationary operands) go out on the scalar HWDGE queue:
            # the sync ring then issues w slab triggers back-to-back from the
            # first kernel instruction, starting the weight stream ~0.9us
            # earlier. x interleaves with slab 0 on the shared engines and
            # still lands long before the first matmul needs it. (x is not
            # compute-gated, so it cannot block anything through scalar's
            # sem lanes the way compute-gated y stores would.)
            EC = EPC * C
            xt_all = xp.tile([P, I_CHUNKS * EC], x_dt, tag="x", name="x")
            nc.scalar.dma_start(out=xt_all[:], in_=xT[:])
            xtiles = [xt_all[:, i * EC:(i + 1) * EC] for i in range(I_CHUNKS)]

            def issue_slab(k):
                wt = wp.tile([P, SLAB_COLS], w_dt, tag="w0",
                             name=f"w{k}", bufs=wbufs)
                if k == SLABS - 1 and EPS > 1:
                    # final slab split per expert — and the very last expert
                    # per i-chunk — so the tail after the last weight byte is
                    # one i-chunk's matmul wave, not a whole expert. Matmuls
                    # release as each piece lands. (Splitting earlier slabs
                    # measured worse — completion receipts pace the trigger
                    # FIFO — but nothing queues behind these.)
                    ecols = I_CHUNKS * H
                    for e in range(EPS - 1):
                        nc.sync.dma_start(out=wt[:, e * ecols:(e + 1) * ecols],
                                          in_=wc[k, :, e * ecols:(e + 1) * ecols])
                    lo = (EPS - 1) * ecols
                    for i in range(I_CHUNKS):
                        nc.sync.dma_start(
                            out=wt[:, lo + i * H:lo + (i + 1) * H],
                            in_=wc[k, :, lo + i * H:lo + (i + 1) * H])
                else:
                    nc.sync.dma_start(out=wt[:], in_=wc[k])
                return wt

            # issue every slab DMA upfront when all buffers are resident
            # (fp8: 8 x 2 MiB); otherwise stream with buffer rotation
            wts = {k: issue_slab(k) for k in range(min(wbufs, SLABS))}

            for b in range(EPC):
                k = b // EPS
                if k not in wts:
                    wts[k] = issue_slab(k)
                wt = wts[k]
                wo = (b % EPS) * I_CHUNKS * H
                ps = pp.tile([PPART, NB * H_CHUNK], mybir.dt.float32,
                             tag="ps", name=f"ps{b}")
                for i in range(I_CHUNKS):
                    for h in range(H_CHUNKS):
                        g, bank = h % G, h // G
                        nc.tensor.matmul(
                            ps[g * C:(g + 1) * C,
                               bank * H_CHUNK:(bank + 1) * H_CHUNK],
                            lhsT=xtiles[i][:, b * C:(b + 1) * C],
                            rhs=wt[:, wo + i * H + h * H_CHUNK:
                                   wo + i * H + (h + 1) * H_CHUNK],
                            start=(i == 0),
                            stop=(i == I_CHUNKS - 1),
                            tile_position=(0, g * C) if G > 1 else None,
                        )
                ot = op.tile([PPART, NB * H_CHUNK], y_dt, tag="o", name=f"o{b}")
                nc.vector.tensor_copy(out=ot[:], in_=ps[:])
                if ns is not None:
                    # only the ns[b] valid rows of each of the G partition
                    # groups move: y bytes drop ~2x, directly off the shared
                    # DMA engines that also carry the weight stream. One DMA
                    # per group — SBUF APs can only treat dim 0 as the
                    # partition dim, so a nested (g c) partition pattern in a
                    # single DMA silently misaddresses rows past the first.
                    # These ride the scalar HWDGE queue (behind x): HWDGE
                    # trigger cost scales with descriptor lines (~6ns/line),
                    # so a ~20-line store is ~10x cheaper to issue than via
                    # gpsimd SWDGE, whose ~0.9us/DMA ucode dispatch made 64
                    # small stores serialize past the end of the w stream.
                    n = max(1, min(int(ns[b]), C))
                    for g in range(G):
                        nc.scalar.dma_start(
                            out=y[b][g * C:g * C + n, :],
                            in_=ot[g * C:g * C + n, :])
                else:
                    # full-tile fallback via SWDGE (gpsimd): separate
                    # descriptor path, can't block the HWDGE trigger FIFOs
                    nc.gpsimd.dma_start(out=y[b], in_=ot[:])
    nc.compile()
    return nc


def _get_nc(C: int, dt_name: str, ns: tuple | None = None):
    key = (C, dt_name, ns)
    if key not in _cache:
        _cache[key] = _build(C, dt_name, ns)
    return _cache[key]


def _np_dt(name):
    import ml_dtypes
    return {
        "float8e3": ml_dtypes.float8_e3m4,
        "bfloat16": ml_dtypes.bfloat16,
        "float32": np.float32,
        "float32r": np.float32,
    }[name]


def _prepare(x, w, chosen_experts, expert_weight, dt_name):
    """Host-side routing. Returns (C, ns, in_maps, row_lists) where
    row_lists[c][s] is the array of global row ids for core c, expert slot
    s, and ns[s] the per-slot valid row count baked into the kernel."""
    w_dtn, x_dtn, _ = DT_CONFIGS[dt_name]
    x = np.asarray(x, dtype=np.float32)
    w = np.asarray(w, dtype=np.float32)
    ce = np.asarray(chosen_experts).astype(np.int64).reshape(-1)      # [T*K]
    gw = np.asarray(expert_weight, dtype=np.float32).reshape(-1)      # [T*K]

    counts = np.bincount(ce, minlength=E)
    C = max(32, int(np.ceil(counts.max() / 32.0) * 32))

    order = np.argsort(ce, kind="stable")
    starts = np.zeros(E + 1, dtype=np.int64)
    np.cumsum(counts, out=starts[1:])

    xs = x * gw[:, None]  # fold router gate into rows (fp32)

    if w_dtn == "float8e3":
        # per-expert scale into the e3m4 range; inverse folded into x rows
        s = E3M4_SCALE_TARGET / np.maximum(
            np.abs(w).max(axis=(1, 2)), 1e-30)                        # [E]
    else:
        s = np.ones(E, dtype=np.float32)

    # Assign experts to (core, slot) in count-sorted rank groups: slot b on
    # every core gets an expert of rank group b, so one per-slot row count
    # (the group max) is tight for the whole SPMD program, y stores move
    # only real rows, and per-core load balances.
    rank = np.argsort(-counts, kind="stable")          # expert ids, big first
    assign = rank.reshape(EPC, N_CORES)                # [slot, core]
    ns = tuple(int(counts[assign[b]].max()) for b in range(EPC))

    EPS = _eps(w_dtn)
    in_maps, row_lists = [], []
    for c in range(N_CORES):
        xg = np.zeros((EPC * C, I_DIM), dtype=np.float32)
        rows_c = []
        for sl in range(EPC):
            e = int(assign[sl, c])
            rows = order[starts[e]:starts[e + 1]]
            xg[sl * C: sl * C + len(rows)] = xs[rows] * (1.0 / s[e])
            rows_c.append(rows)
        # [b, i*128+p, h] -> [k, p, e*ICH*H + i*H + h] (b = k*EPS+e):
        # contiguous per-partition slab lines, EPS experts per DMA slab
        eids = assign[:, c]
        wcore = (
            (w[eids] * s[eids, None, None])
            .reshape(EPC // EPS, EPS, I_CHUNKS, P, H)
            .transpose(0, 3, 1, 2, 4)
            .reshape(EPC // EPS, P, EPS * I_CHUNKS * H)
        )
        # [c, i*128+p] -> [p, i*EC + c]: one resident stationary tile
        xre = (
            xg.reshape(EPC * C, I_CHUNKS, P)
            .transpose(2, 1, 0)
            .reshape(P, I_CHUNKS * EPC * C)
        )
        in_maps.append({
            "wc": np.ascontiguousarray(wcore).astype(_np_dt(w_dtn)),
            "xT": np.ascontiguousarray(xre).astype(_np_dt(x_dtn)),
        })
        row_lists.append(rows_c)
    return C, ns, in_maps, row_lists


def _combine(results, row_lists, C, dt_name):
    G = max(1, P // C)
    if H_CHUNKS % G != 0 or DT_CONFIGS[dt_name][0] not in ("bfloat16", "float8e3"):
        G = 1
    NB = H_CHUNKS // G
    yfull = np.empty((T * K_TOP, H), dtype=np.float32)
    for c in range(N_CORES):
        yc = np.asarray(results[c]["y"], dtype=np.float32)  # [EPC, G*C, NB*512]
        # unpack: partition (g*C+r), col (bank*512+hc) -> out[r, (bank*G+g)*512+hc]
        yc = yc.reshape(EPC, G, C, NB, H_CHUNK).transpose(0, 2, 3, 1, 4).reshape(EPC, C, H)
        for s, rows in enumerate(row_lists[c]):
            if len(rows):
                yfull[rows] = yc[s, : len(rows)]
    return yfull[0::2] + yfull[1::2]


def run(x, w, chosen_experts, expert_weight, dt_name=DEFAULT_DTYPE, **spmd_kwargs):
    from concourse.bass_utils import run_bass_kernel_spmd

    C, ns, in_maps, row_lists = _prepare(x, w, chosen_experts, expert_weight, dt_name)
    nc = _get_nc(C, dt_name, ns)
    res = run_bass_kernel_spmd(nc, in_maps, core_ids=list(range(N_CORES)), **spmd_kwargs)
    out = _combine(res.results, row_lists, C, dt_name)
    return out, res


def kernel(x, w, chosen_experts, expert_weight):
    out, _ = run(x, w, chosen_experts, expert_weight)
    return out



# revision 9
# speedup vs baseline: 1.1445x; 1.1445x over previous
"""MoE down-projection (grouped GEMM + topk combine) on 8 Trainium2 cores.

Strategy: expert-parallel. Each of the 8 cores owns E/8 = 16 experts and
receives (a) its experts' weight slabs and (b) the x rows routed to those
experts, gathered+gate-scaled+transposed on host, padded per expert to a
fixed capacity C. The device kernel is a block-diagonal grouped GEMM.
Weights stream through the PE as the moving operand (full rate); the few
x rows per expert are the stationary operand. NG = 128//C experts are
processed concurrently in separate PE column groups (tile_position), each
owning a contiguous C-partition range of a [128, H] PSUM tile, so output
stores are plain partition slices of only the real token rows. Host
scatter-adds the row results back into the [T, H] output.

The kernel is HBM-bandwidth bound on the weight stream, so the default
config stores w as fp8 E3M4 (per-expert scale folded into the x rows) and
the y output as bf16 — halving both read and write traffic vs bf16/f32 at
~1.3e-2 relative error (gate is 2e-2). Experts are assigned to (core,
slot) in count-sorted rank groups so one per-slot row count is tight for
the whole SPMD program.

Hardcoded problem shape (from the problem spec):
  x: [2048, 512] f32, w: [128, 512, 2048] f32,
  chosen_experts: [1024, 2] int, expert_weight: [1024, 2] f32 -> out [1024, 2048] f32
"""

import numpy as np

T = 1024
K_TOP = 2
E = 128
I_DIM = 512
H = 2048
N_CORES = 8
EPC = E // N_CORES  # experts per core = 16
P = 128             # partitions
I_CHUNKS = I_DIM // P       # 4
H_CHUNK = 512               # matmul moving free dim (fp32 PSUM bank)
H_CHUNKS = H // H_CHUNK     # 4

# matmul dtype config: name -> (w dtype, x dtype, y dtype)
#   float8e3  : w E3M4 (per-expert scaled), x bf16, y bf16 — half DMA traffic
#   float8e3x : both operands E3M4 (if mixed-dtype matmul is unsupported)
#   bfloat16  : both bf16, y f32
#   float32 / float32r: exact / reduced-precision f32
DT_CONFIGS = {
    "float8e3": ("float8e3", "bfloat16", "bfloat16"),
    "float8e3x": ("float8e3", "float8e3", "bfloat16"),
    "bfloat16": ("bfloat16", "bfloat16", "float32"),
    "float32": ("float32", "float32", "float32"),
    "float32r": ("float32r", "float32r", "float32"),
}
DEFAULT_DTYPE = "float8e3"
E3M4_SCALE_TARGET = 14.0  # keep clear of the 15.5 e3m4 max normal

_cache = {}


def _w_bytes(w_dtn):
    return 1 if w_dtn == "float8e3" else (2 if w_dtn == "bfloat16" else 4)


def _eps(w_dtn):
    """Experts per DMA slab. 2 MiB transfers amortize the per-DMA completion
    receipt (~1-2 us) that gates the next trigger on the same sem lane while
    keeping per-expert compute release fine-grained; bigger slabs quantize
    the compute pipeline and pay larger receipt latencies (measured loss)."""
    return max(1, 2 * 1024 * 1024 // (P * I_CHUNKS * H * _w_bytes(w_dtn)))


def _build(C: int, dt_name: str, ns: tuple | None = None):
    """ns: per-slot valid row counts (same for every core by construction —
    the host assigns experts to slots in count-sorted rank groups). When
    given, y stores move only those rows."""
    import concourse.mybir as mybir
    import concourse.tile as tile
    from concourse import bacc

    w_dtn, x_dtn, y_dtn = DT_CONFIGS[dt_name]
    w_dt = getattr(mybir.dt, w_dtn)
    x_dt = getattr(mybir.dt, x_dtn)
    y_dt = getattr(mybir.dt, y_dtn)
    w_bytes = _w_bytes(w_dtn)
    EPS = _eps(w_dtn)
    SLABS = EPC // EPS
    SLAB_COLS = EPS * I_CHUNKS * H
    # G = 128//C PE column groups run one expert's G h-chunks concurrently;
    # expert b's H chunk h goes to psum partitions (h%G)*C..+C, bank cols
    # (h//G)*512..+512, so casts and stores use all 128 partitions. Host
    # unpacks. fp32 rejects tile_position col-tiling.
    G = max(1, P // C)
    if H_CHUNKS % G != 0 or w_dtn not in ("bfloat16", "float8e3"):
        G = 1
    NB = H_CHUNKS // G
    PPART = G * C
    # keep the whole weight working set resident when it fits (fp8: 16 MiB)
    wbufs = SLABS if w_bytes == 1 else (6 if w_bytes == 2 else 3)
    obufs = 4 if w_bytes <= 2 else 2
    pbufs = 6 if NB == 1 else 2

    nc = bacc.Bacc()
    # wc host-prearranged: [k, p, e*ICH*H + i*H + h] = w[k*EPS+e, i*128+p, h]
    # so each partition's slab line is 1 contiguous run per DMA
    wc = nc.declare_dram_parameter("wc", [SLABS, P, SLAB_COLS], w_dt, isOutput=False)
    # x host-prearranged: [p, i*EC + c] = x[i*128+p, c] (EC = EPC*C) so the
    # whole stationary operand arrives in ONE small DMA before the w flood
    xT = nc.declare_dram_parameter("xT", [P, I_CHUNKS * EPC * C], x_dt, isOutput=False)
    # y rows are stored compacted: y[b, r] = full H row for valid row r < ns[b]
    y = nc.declare_dram_parameter("y", [EPC, C, H], y_dt, isOutput=True)

    with tile.TileContext(nc) as tc:
        with (
            tc.tile_pool(name="wp", bufs=wbufs) as wp,
            tc.tile_pool(name="xp", bufs=1) as xp,
            tc.tile_pool(name="pp", bufs=pbufs, space="PSUM") as pp,
            tc.tile_pool(name="op", bufs=obufs) as op,
        ):
            # x rows (stationary operands) go out on the scalar HWDGE queue:
            # the sync ring then issues w slab triggers back-to-back from the
            # first kernel instruction, starting the weight stream ~0.9us
            # earlier. x interleaves with slab 0 on the shared engines and
            # still lands long before the first matmul needs it. (x is not
            # compute-gated, so it cannot block anything through scalar's
            # sem lanes the way compute-gated y stores would.)
            EC = EPC * C
            xt_all = xp.tile([P, I_CHUNKS * EC], x_dt, tag="x", name="x")
            nc.scalar.dma_start(out=xt_all[:], in_=xT[:])
            xtiles = [xt_all[:, i * EC:(i + 1) * EC] for i in range(I_CHUNKS)]

            def issue_slab(k):
                wt = wp.tile([P, SLAB_COLS], w_dt, tag="w0",
                             name=f"w{k}", bufs=wbufs)
                if k == SLABS - 1 and EPS > 1:
                    # final slab split per expert — and the very last expert
                    # per i-chunk — so the tail after the last weight byte is
                    # one i-chunk's matmul wave, not a whole expert. Matmuls
                    # release as each piece lands. (Splitting earlier slabs
                    # measured worse — completion receipts pace the trigger
                    # FIFO — but nothing queues behind these.)
                    ecols = I_CHUNKS * H
                    for e in range(EPS - 1):
                        nc.sync.dma_start(out=wt[:, e * ecols:(e + 1) * ecols],
                                          in_=wc[k, :, e * ecols:(e + 1) * ecols])
                    lo = (EPS - 1) * ecols
                    for i in range(I_CHUNKS):
                        nc.sync.dma_start(
                            out=wt[:, lo + i * H:lo + (i + 1) * H],
                            in_=wc[k, :, lo + i * H:lo + (i + 1) * H])
                else:
                    nc.sync.dma_start(out=wt[:], in_=wc[k])
                return wt

            # issue every slab DMA upfront when all buffers are resident
            # (fp8: 8 x 2 MiB); otherwise stream with buffer rotation
            wts = {k: issue_slab(k) for k in range(min(wbufs, SLABS))}

            for b in range(EPC):
                k = b // EPS
                if k not in wts:
                    wts[k] = issue_slab(k)
                wt = wts[k]
                wo = (b % EPS) * I_CHUNKS * H
                ps = pp.tile([PPART, NB * H_CHUNK], mybir.dt.float32,
                             tag="ps", name=f"ps{b}")
                for i in range(I_CHUNKS):
                    for h in range(H_CHUNKS):
                        g, bank = h % G, h // G
                        nc.tensor.matmul(
                            ps[g * C:(g + 1) * C,
                               bank * H_CHUNK:(bank + 1) * H_CHUNK],
                            lhsT=xtiles[i][:, b * C:(b + 1) * C],
                            rhs=wt[:, wo + i * H + h * H_CHUNK:
                                   wo + i * H + (h + 1) * H_CHUNK],
                            start=(i == 0),
                            stop=(i == I_CHUNKS - 1),
                            tile_position=(0, g * C) if G > 1 else None,
                        )
                # Compact the G partition groups' valid rows into the FREE
                # dim during the psum->sbuf cast: copy (g, bank) block
                # ps[g*C : g*C+n, bank*512:+512] -> ot[0:n, (bank*G+g)*512],
                # giving a row-major [n, H] tile. Partition bases stay
                # 32-aligned (engine ops reject unaligned bases) and the
                # expert then needs ONE y store of only its n valid rows —
                # y bytes drop ~2x off the shared DMA engines that also
                # carry the w stream. One DMA per expert matters: both DGE
                # paths cost ~0.8us of queue time per trigger regardless of
                # size, so per-group stores (4x count) serialize past the
                # stream end (measured +20us on either gpsimd or scalar).
                # Copies alternate vector/scalar; either alone can become
                # the pacing engine at ~0.7us per 512-col block.
                n = C if ns is None else max(1, min(int(ns[b]), C))
                ot = op.tile([C, H], y_dt, tag="o", name=f"o{b}")
                for g in range(G):
                    for bank in range(NB):
                        eng = nc.vector if (g * NB + bank) % 2 == 0 else nc.scalar
                        dst = ot[0:n, (bank * G + g) * H_CHUNK:
                                 (bank * G + g + 1) * H_CHUNK]
                        src = ps[g * C:g * C + n,
                                 bank * H_CHUNK:(bank + 1) * H_CHUNK]
                        if eng is nc.vector:
                            eng.tensor_copy(out=dst, in_=src)
                        else:
                            eng.copy(out=dst, in_=src)
                # y stores go out via SWDGE (gpsimd): its descriptor path and
                # completion semaphores are separate from the HWDGE rings, so
                # slow compute-gated stores can never block the weight-slab
                # trigger FIFO through shared sem lanes (observed with both
                # stores-on-sync and stores-on-scalar)
                nc.gpsimd.dma_start(out=y[b, 0:n, :], in_=ot[0:n, :])
    nc.compile()
    return nc


def _get_nc(C: int, dt_name: str, ns: tuple | None = None):
    key = (C, dt_name, ns)
    if key not in _cache:
        _cache[key] = _build(C, dt_name, ns)
    return _cache[key]


def _np_dt(name):
    import ml_dtypes
    return {
        "float8e3": ml_dtypes.float8_e3m4,
        "bfloat16": ml_dtypes.bfloat16,
        "float32": np.float32,
        "float32r": np.float32,
    }[name]


def _prepare(x, w, chosen_experts, expert_weight, dt_name):
    """Host-side routing. Returns (C, ns, in_maps, row_lists) where
    row_lists[c][s] is the array of global row ids for core c, expert slot
    s, and ns[s] the per-slot valid row count baked into the kernel."""
    w_dtn, x_dtn, _ = DT_CONFIGS[dt_name]
    x = np.asarray(x, dtype=np.float32)
    w = np.asarray(w, dtype=np.float32)
    ce = np.asarray(chosen_experts).astype(np.int64).reshape(-1)      # [T*K]
    gw = np.asarray(expert_weight, dtype=np.float32).reshape(-1)      # [T*K]

    counts = np.bincount(ce, minlength=E)
    C = max(32, int(np.ceil(counts.max() / 32.0) * 32))

    order = np.argsort(ce, kind="stable")
    starts = np.zeros(E + 1, dtype=np.int64)
    np.cumsum(counts, out=starts[1:])

    xs = x * gw[:, None]  # fold router gate into rows (fp32)

    if w_dtn == "float8e3":
        # per-expert scale into the e3m4 range; inverse folded into x rows
        s = E3M4_SCALE_TARGET / np.maximum(
            np.abs(w).max(axis=(1, 2)), 1e-30)                        # [E]
    else:
        s = np.ones(E, dtype=np.float32)

    # Assign experts to (core, slot) in count-sorted rank groups: slot b on
    # every core gets an expert of rank group b, so one per-slot row count
    # (the group max) is tight for the whole SPMD program, y stores move
    # only real rows, and per-core load balances.
    rank = np.argsort(-counts, kind="stable")          # expert ids, big first
    assign = rank.reshape(EPC, N_CORES)                # [slot, core]
    ns = tuple(int(counts[assign[b]].max()) for b in range(EPC))

    EPS = _eps(w_dtn)
    in_maps, row_lists = [], []
    for c in range(N_CORES):
        xg = np.zeros((EPC * C, I_DIM), dtype=np.float32)
        rows_c = []
        for sl in range(EPC):
            e = int(assign[sl, c])
            rows = order[starts[e]:starts[e + 1]]
            xg[sl * C: sl * C + len(rows)] = xs[rows] * (1.0 / s[e])
            rows_c.append(rows)
        # [b, i*128+p, h] -> [k, p, e*ICH*H + i*H + h] (b = k*EPS+e):
        # contiguous per-partition slab lines, EPS experts per DMA slab
        eids = assign[:, c]
        wcore = (
            (w[eids] * s[eids, None, None])
            .reshape(EPC // EPS, EPS, I_CHUNKS, P, H)
            .transpose(0, 3, 1, 2, 4)
            .reshape(EPC // EPS, P, EPS * I_CHUNKS * H)
        )
        # [c, i*128+p] -> [p, i*EC + c]: one resident stationary tile
        xre = (
            xg.reshape(EPC * C, I_CHUNKS, P)
            .transpose(2, 1, 0)
            .reshape(P, I_CHUNKS * EPC * C)
        )
        in_maps.append({
            "wc": np.ascontiguousarray(wcore).astype(_np_dt(w_dtn)),
            "xT": np.ascontiguousarray(xre).astype(_np_dt(x_dtn)),
        })
        row_lists.append(rows_c)
    return C, ns, in_maps, row_lists


def _combine(results, row_lists, C, dt_name):
    # device stores row-major [n, H] per expert slot (compacted valid rows)
    yfull = np.empty((T * K_TOP, H), dtype=np.float32)
    for c in range(N_CORES):
        yc = np.asarray(results[c]["y"], dtype=np.float32)  # [EPC, C, H]
        for s, rows in enumerate(row_lists[c]):
            if len(rows):
                yfull[rows] = yc[s, : len(rows)]
    return yfull[0::2] + yfull[1::2]


def run(x, w, chosen_experts, expert_weight, dt_name=DEFAULT_DTYPE, **spmd_kwargs):
    from concourse.bass_utils import run_bass_kernel_spmd

    C, ns, in_maps, row_lists = _prepare(x, w, chosen_experts, expert_weight, dt_name)
    nc = _get_nc(C, dt_name, ns)
    res = run_bass_kernel_spmd(nc, in_maps, core_ids=list(range(N_CORES)), **spmd_kwargs)
    out = _combine(res.results, row_lists, C, dt_name)
    return out, res


def kernel(x, w, chosen_experts, expert_weight):
    out, _ = run(x, w, chosen_experts, expert_weight)
    return out



# revision 12
# speedup vs baseline: 1.1991x; 1.0477x over previous
"""MoE down-projection (grouped GEMM + topk combine) on 8 Trainium2 cores.

Strategy: expert-parallel. Each of the 8 cores owns E/8 = 16 experts and
receives (a) its experts' weight slabs and (b) the x rows routed to those
experts, gathered+gate-scaled+transposed on host, padded per expert to a
fixed capacity C. The device kernel is a block-diagonal grouped GEMM.
Weights stream through the PE as the moving operand (full rate); the few
x rows per expert are the stationary operand. NG = 128//C experts are
processed concurrently in separate PE column groups (tile_position), each
owning a contiguous C-partition range of a [128, H] PSUM tile, so output
stores are plain partition slices of only the real token rows. Host
scatter-adds the row results back into the [T, H] output.

The kernel is HBM-bandwidth bound on the weight stream, so the default
config stores w as fp8 E3M4 (per-expert scale folded into the x rows) and
the y output as bf16 — halving both read and write traffic vs bf16/f32 at
~1.3e-2 relative error (gate is 2e-2). Experts are assigned to (core,
slot) in count-sorted rank groups so one per-slot row count is tight for
the whole SPMD program.

Hardcoded problem shape (from the problem spec):
  x: [2048, 512] f32, w: [128, 512, 2048] f32,
  chosen_experts: [1024, 2] int, expert_weight: [1024, 2] f32 -> out [1024, 2048] f32
"""

import numpy as np

T = 1024
K_TOP = 2
E = 128
I_DIM = 512
H = 2048
N_CORES = 8
EPC = E // N_CORES  # experts per core = 16
P = 128             # partitions
I_CHUNKS = I_DIM // P       # 4
H_CHUNK = 512               # matmul moving free dim (fp32 PSUM bank)
H_CHUNKS = H // H_CHUNK     # 4

# matmul dtype config: name -> (w dtype, x dtype, y dtype)
#   float8e3  : w E3M4 (per-expert scaled), x bf16, y bf16 — half DMA traffic
#   float8e3x : both operands E3M4 (if mixed-dtype matmul is unsupported)
#   bfloat16  : both bf16, y f32
#   float32 / float32r: exact / reduced-precision f32
DT_CONFIGS = {
    "float8e3": ("float8e3", "bfloat16", "bfloat16"),
    "float8e3x": ("float8e3", "float8e3", "bfloat16"),
    "bfloat16": ("bfloat16", "bfloat16", "float32"),
    "float32": ("float32", "float32", "float32"),
    "float32r": ("float32r", "float32r", "float32"),
}
DEFAULT_DTYPE = "float8e3"
E3M4_SCALE_TARGET = 14.0  # keep clear of the 15.5 e3m4 max normal

_cache = {}


def _w_bytes(w_dtn):
    return 1 if w_dtn == "float8e3" else (2 if w_dtn == "bfloat16" else 4)


def _eps(w_dtn):
    """Experts per DMA slab. 2 MiB transfers amortize the per-DMA completion
    receipt (~1-2 us) that gates the next trigger on the same sem lane while
    keeping per-expert compute release fine-grained; bigger slabs quantize
    the compute pipeline and pay larger receipt latencies (measured loss)."""
    return max(1, 2 * 1024 * 1024 // (P * I_CHUNKS * H * _w_bytes(w_dtn)))


def _build(C: int, dt_name: str, ns: tuple | None = None):
    """ns: per-slot valid row counts (same for every core by construction —
    the host assigns experts to slots in count-sorted rank groups). When
    given, y stores move only those rows."""
    import concourse.mybir as mybir
    import concourse.tile as tile
    from concourse import bacc

    w_dtn, x_dtn, y_dtn = DT_CONFIGS[dt_name]
    w_dt = getattr(mybir.dt, w_dtn)
    x_dt = getattr(mybir.dt, x_dtn)
    y_dt = getattr(mybir.dt, y_dtn)
    w_bytes = _w_bytes(w_dtn)
    EPS = _eps(w_dtn)
    SLABS = EPC // EPS
    SLAB_COLS = EPS * I_CHUNKS * H
    # G = 128//C PE column groups run one expert's G h-chunks concurrently;
    # expert b's H chunk h goes to psum partitions (h%G)*C..+C, bank cols
    # (h//G)*512..+512, so casts and stores use all 128 partitions. Host
    # unpacks. fp32 rejects tile_position col-tiling.
    G = max(1, P // C)
    if H_CHUNKS % G != 0 or w_dtn not in ("bfloat16", "float8e3"):
        G = 1
    NB = H_CHUNKS // G
    PPART = G * C
    # keep the whole weight working set resident when it fits (fp8: 16 MiB)
    wbufs = SLABS if w_bytes == 1 else (6 if w_bytes == 2 else 3)
    # ot tiles are small ([C, H] y_dt); deep rotation keeps the cast->store
    # WAR chain from ever pacing the compute pipeline on store latency
    obufs = 8 if w_bytes <= 2 else 2
    pbufs = 6 if NB == 1 else 2

    nc = bacc.Bacc()
    # wc host-prearranged: [k, p, e*ICH*H + i*H + h] = w[k*EPS+e, i*128+p, h]
    # so each partition's slab line is 1 contiguous run per DMA
    wc = nc.declare_dram_parameter("wc", [SLABS, P, SLAB_COLS], w_dt, isOutput=False)
    # x host-prearranged: [p, i*EC + c] = x[i*128+p, c] (EC = EPC*C) so the
    # whole stationary operand arrives in ONE small DMA before the w flood
    xT = nc.declare_dram_parameter("xT", [P, I_CHUNKS * EPC * C], x_dt, isOutput=False)
    # y rows are stored compacted: y[b, r] = full H row for valid row r < ns[b]
    y = nc.declare_dram_parameter("y", [EPC, C, H], y_dt, isOutput=True)

    with tile.TileContext(nc) as tc:
        with (
            tc.tile_pool(name="wp", bufs=wbufs) as wp,
            tc.tile_pool(name="xp", bufs=1) as xp,
            tc.tile_pool(name="pp", bufs=pbufs, space="PSUM") as pp,
            tc.tile_pool(name="op", bufs=obufs) as op,
        ):
            # x rows (stationary operands) go out on the scalar HWDGE queue:
            # the sync ring then issues w slab triggers back-to-back from the
            # first kernel instruction, starting the weight stream ~0.9us
            # earlier. x interleaves with slab 0 on the shared engines and
            # still lands long before the first matmul needs it. (x is not
            # compute-gated, so it cannot block anything through scalar's
            # sem lanes the way compute-gated y stores would.)
            EC = EPC * C
            xt_all = xp.tile([P, I_CHUNKS * EC], x_dt, tag="x", name="x")
            nc.scalar.dma_start(out=xt_all[:], in_=xT[:])
            xtiles = [xt_all[:, i * EC:(i + 1) * EC] for i in range(I_CHUNKS)]

            def issue_slab(k):
                wt = wp.tile([P, SLAB_COLS], w_dt, tag="w0",
                             name=f"w{k}", bufs=wbufs)
                if k == SLABS - 1 and EPS > 1:
                    # final slab split per expert — and the very last expert
                    # per i-chunk — so the tail after the last weight byte is
                    # one i-chunk's matmul wave, not a whole expert. Matmuls
                    # release as each piece lands. (Splitting earlier slabs
                    # measured worse — completion receipts pace the trigger
                    # FIFO — but nothing queues behind these.)
                    ecols = I_CHUNKS * H
                    for e in range(EPS - 1):
                        nc.sync.dma_start(out=wt[:, e * ecols:(e + 1) * ecols],
                                          in_=wc[k, :, e * ecols:(e + 1) * ecols])
                    lo = (EPS - 1) * ecols
                    for i in range(I_CHUNKS):
                        nc.sync.dma_start(
                            out=wt[:, lo + i * H:lo + (i + 1) * H],
                            in_=wc[k, :, lo + i * H:lo + (i + 1) * H])
                else:
                    nc.sync.dma_start(out=wt[:], in_=wc[k])
                return wt

            # issue every slab DMA upfront when all buffers are resident
            # (fp8: 8 x 2 MiB); otherwise stream with buffer rotation
            wts = {k: issue_slab(k) for k in range(min(wbufs, SLABS))}

            for b in range(EPC):
                k = b // EPS
                if k not in wts:
                    wts[k] = issue_slab(k)
                wt = wts[k]
                wo = (b % EPS) * I_CHUNKS * H
                ps = pp.tile([PPART, NB * H_CHUNK], mybir.dt.float32,
                             tag="ps", name=f"ps{b}")
                for i in range(I_CHUNKS):
                    for h in range(H_CHUNKS):
                        g, bank = h % G, h // G
                        nc.tensor.matmul(
                            ps[g * C:(g + 1) * C,
                               bank * H_CHUNK:(bank + 1) * H_CHUNK],
                            lhsT=xtiles[i][:, b * C:(b + 1) * C],
                            rhs=wt[:, wo + i * H + h * H_CHUNK:
                                   wo + i * H + (h + 1) * H_CHUNK],
                            start=(i == 0),
                            stop=(i == I_CHUNKS - 1),
                            tile_position=(0, g * C) if G > 1 else None,
                        )
                # Compact the G partition groups' valid rows into the FREE
                # dim during the psum->sbuf cast: copy (g, bank) block
                # ps[g*C : g*C+n, bank*512:+512] -> ot[0:n, (bank*G+g)*512],
                # giving a row-major [n, H] tile. Partition bases stay
                # 32-aligned (engine ops reject unaligned bases) and the
                # expert then needs ONE y store of only its n valid rows —
                # y bytes drop ~2x off the shared DMA engines that also
                # carry the w stream. One DMA per expert matters: both DGE
                # paths cost ~0.8us of queue time per trigger regardless of
                # size, so per-group stores (4x count) serialize past the
                # stream end (measured +20us on either gpsimd or scalar).
                # Copies alternate vector/scalar; either alone can become
                # the pacing engine at ~0.7us per 512-col block.
                n = C if ns is None else max(1, min(int(ns[b]), C))
                ot = op.tile([C, H], y_dt, tag="o", name=f"o{b}")
                for g in range(G):
                    for bank in range(NB):
                        eng = nc.vector if (g * NB + bank) % 2 == 0 else nc.scalar
                        dst = ot[0:n, (bank * G + g) * H_CHUNK:
                                 (bank * G + g + 1) * H_CHUNK]
                        src = ps[g * C:g * C + n,
                                 bank * H_CHUNK:(bank + 1) * H_CHUNK]
                        if eng is nc.vector:
                            eng.tensor_copy(out=dst, in_=src)
                        else:
                            eng.copy(out=dst, in_=src)
                # y stores ride the sync HWDGE queue when all w slab
                # triggers are issued upfront (wbufs == SLABS, the fp8
                # path): the queue is idle after ~17us and a compute-gated
                # store waiting at its head blocks nothing. gpsimd SWDGE is
                # NOT usable here: its ucode spends ~2.2us per store
                # generating small packets, which paced the whole tail at
                # 2.2us/expert through the ot WAR chain (measured +15us).
                # With slab-buffer rotation (wbufs < SLABS) a waiting store
                # WOULD stall later slab triggers, so fall back to SWDGE.
                if wbufs == SLABS:
                    nc.sync.dma_start(out=y[b, 0:n, :], in_=ot[0:n, :])
                else:
                    nc.gpsimd.dma_start(out=y[b, 0:n, :], in_=ot[0:n, :])
    nc.compile()
    return nc


def _get_nc(C: int, dt_name: str, ns: tuple | None = None):
    key = (C, dt_name, ns)
    if key not in _cache:
        _cache[key] = _build(C, dt_name, ns)
    return _cache[key]


def _np_dt(name):
    import ml_dtypes
    return {
        "float8e3": ml_dtypes.float8_e3m4,
        "bfloat16": ml_dtypes.bfloat16,
        "float32": np.float32,
        "float32r": np.float32,
    }[name]


def _prepare(x, w, chosen_experts, expert_weight, dt_name):
    """Host-side routing. Returns (C, ns, in_maps, row_lists) where
    row_lists[c][s] is the array of global row ids for core c, expert slot
    s, and ns[s] the per-slot valid row count baked into the kernel."""
    w_dtn, x_dtn, _ = DT_CONFIGS[dt_name]
    x = np.asarray(x, dtype=np.float32)
    w = np.asarray(w, dtype=np.float32)
    ce = np.asarray(chosen_experts).astype(np.int64).reshape(-1)      # [T*K]
    gw = np.asarray(expert_weight, dtype=np.float32).reshape(-1)      # [T*K]

    counts = np.bincount(ce, minlength=E)
    C = max(32, int(np.ceil(counts.max() / 32.0) * 32))

    order = np.argsort(ce, kind="stable")
    starts = np.zeros(E + 1, dtype=np.int64)
    np.cumsum(counts, out=starts[1:])

    xs = x * gw[:, None]  # fold router gate into rows (fp32)

    if w_dtn == "float8e3":
        # per-expert scale into the e3m4 range; inverse folded into x rows
        s = E3M4_SCALE_TARGET / np.maximum(
            np.abs(w).max(axis=(1, 2)), 1e-30)                        # [E]
    else:
        s = np.ones(E, dtype=np.float32)

    # Assign experts to (core, slot) in count-sorted rank groups: slot b on
    # every core gets an expert of rank group b, so one per-slot row count
    # (the group max) is tight for the whole SPMD program, y stores move
    # only real rows, and per-core load balances.
    rank = np.argsort(-counts, kind="stable")          # expert ids, big first
    assign = rank.reshape(EPC, N_CORES)                # [slot, core]
    ns = tuple(int(counts[assign[b]].max()) for b in range(EPC))

    EPS = _eps(w_dtn)
    in_maps, row_lists = [], []
    for c in range(N_CORES):
        xg = np.zeros((EPC * C, I_DIM), dtype=np.float32)
        rows_c = []
        for sl in range(EPC):
            e = int(assign[sl, c])
            rows = order[starts[e]:starts[e + 1]]
            xg[sl * C: sl * C + len(rows)] = xs[rows] * (1.0 / s[e])
            rows_c.append(rows)
        # [b, i*128+p, h] -> [k, p, e*ICH*H + i*H + h] (b = k*EPS+e):
        # contiguous per-partition slab lines, EPS experts per DMA slab
        eids = assign[:, c]
        wcore = (
            (w[eids] * s[eids, None, None])
            .reshape(EPC // EPS, EPS, I_CHUNKS, P, H)
            .transpose(0, 3, 1, 2, 4)
            .reshape(EPC // EPS, P, EPS * I_CHUNKS * H)
        )
        # [c, i*128+p] -> [p, i*EC + c]: one resident stationary tile
        xre = (
            xg.reshape(EPC * C, I_CHUNKS, P)
            .transpose(2, 1, 0)
            .reshape(P, I_CHUNKS * EPC * C)
        )
        in_maps.append({
            "wc": np.ascontiguousarray(wcore).astype(_np_dt(w_dtn)),
            "xT": np.ascontiguousarray(xre).astype(_np_dt(x_dtn)),
        })
        row_lists.append(rows_c)
    return C, ns, in_maps, row_lists


def _combine(results, row_lists, C, dt_name):
    # device stores row-major [n, H] per expert slot (compacted valid rows)
    yfull = np.empty((T * K_TOP, H), dtype=np.float32)
    for c in range(N_CORES):
        yc = np.asarray(results[c]["y"], dtype=np.float32)  # [EPC, C, H]
        for s, rows in enumerate(row_lists[c]):
            if len(rows):
                yfull[rows] = yc[s, : len(rows)]
    return yfull[0::2] + yfull[1::2]


def run(x, w, chosen_experts, expert_weight, dt_name=DEFAULT_DTYPE, **spmd_kwargs):
    from concourse.bass_utils import run_bass_kernel_spmd

    C, ns, in_maps, row_lists = _prepare(x, w, chosen_experts, expert_weight, dt_name)
    nc = _get_nc(C, dt_name, ns)
    res = run_bass_kernel_spmd(nc, in_maps, core_ids=list(range(N_CORES)), **spmd_kwargs)
    out = _combine(res.results, row_lists, C, dt_name)
    return out, res


def kernel(x, w, chosen_experts, expert_weight):
    out, _ = run(x, w, chosen_experts, expert_weight)
    return out

